# revision 9
# baseline (speedup 1.0000x reference)
"""Trainium2 Bass kernel for nn_Classifier_custom_12936441496172.

Reference math (per batch b, with av = column-l2-normalized img_b [Cf, R]):
    A      = softmax_r( (vv @ W1) @ av )          # [I, R] attention over R
    F_p    = A @ av.T                             # [I, Cf]
    out[b] = rowsum( (vv @ W2) * F_p )            # [I]

Key identity used here: out[b, i] = sum_r A[i, r] * ((vv @ W2) @ av)[i, r],
so the big F_p intermediate is never materialized. Both (vv@W1)@av and
(vv@W2)@av come from one stacked weight matrix QPT, and the column
normalization of av folds into a per-column scale of the matmul output:
(Q @ av)[i, r] = (Q @ img_b)[i, r] * rn[r], rn = 1/||img_b[:, r]||.

Sharding: data-parallel over batch across 8 NeuronCores (16 batches each),
with the small parameter matrix QPT replicated. Parameter prep (vv @ W1/W2,
< 1% of total FLOPs) happens on host; all img-dependent compute (norms,
main matmuls, softmax, weighted dots) runs on-device.

Device kernel per core: 8 groups of 2 batches (N = 512 matmul free dim):
  - norms: fp16 squares (ACT/DVE) + fp16 pair-add tree (DVE 2x mode) +
    gpsimd partition_all_reduce -> n2 broadcast on all partitions;
    rn = Exp(-0.5 * Ln(n2)) on ACT (square/ln/exp share one ACT table set,
    so no table reloads).
  - main: 5 m-chunks of the 624 stacked rows (Q0, Q1, P0, P1, QP-tail
    packed into one 112-row chunk via host-side column reorder), each 8
    accumulating float32r matmuls (full PE rate at N >= 256).
  - softmax+dot: S*rn (DVE, reads PSUM), Exp with free per-partition
    accum (ACT) -> sumexp matrix, then one fused DVE scalar_tensor_tensor
    E * S_P with free-axis accum -> unnormalized output column. The
    softmax denominator is applied once per core at the end (3 tiny
    reciprocal+multiply ops on [*, 16] tiles).
Logits are ~N(0,1) (|logit| < ~6) so the softmax max-subtraction is skipped;
exp cannot overflow fp32.
"""

import numpy as np

_PROGRAM = None

# Problem geometry (hardcoded per contract; kernel.py must be self-contained)
N_CORES = 8
NB = 16          # batches per core
R = 256          # H * W
CF = 1024        # feature channels
KC = CF // 128   # 8 contraction chunks
I = 312          # attributes
G = NB // 2      # groups of 2 batches
N = 2 * R        # matmul moving free dim (2 batches)
TQ = I - 256     # 56-row tails
# m-chunk column offsets in the host-reordered qpt
MCH_Q = [0, 128]       # Q rows 0:128, 128:256
MCH_P = [256, 384]     # P rows 0:128, 128:256
MCH_T = 512            # Q rows 256:312 at cols 512:568, P rows at 568:624
# number of squares computed on ACT (rest on DVE)
SQ_ON_ACT = 3


def _build_program():
    import concourse.tile as tile
    from concourse import bacc, bass_isa, mybir

    F32 = mybir.dt.float32
    BF16 = mybir.dt.bfloat16
    F16 = mybir.dt.float16
    MULT = mybir.AluOpType.mult
    EXP = mybir.ActivationFunctionType.Exp
    LN = mybir.ActivationFunctionType.Ln

    nc = bacc.Bacc(
        "TRN2",
        target_bir_lowering=False,
        debug=False,
        enable_asserts=False,
        num_devices=N_CORES,
    )
    img = nc.dram_tensor("img", [G, KC, 128, N], BF16, kind="ExternalInput").ap()
    qpt = nc.dram_tensor("qpt", [CF, 2 * I], BF16, kind="ExternalInput").ap()
    out = nc.dram_tensor("out", [I, NB], F32, kind="ExternalOutput").ap()

    with tile.TileContext(nc) as tc, tc.tile_pool(name="sb", bufs=2) as sb, tc.tile_pool(
        name="ps", bufs=5, space="PSUM"
    ) as ps:
        qpt_sb = sb.tile([128, KC * 2 * I], BF16, tag="qpt", bufs=1, name="qpt_sb")
        for k in range(KC):
            nc.sync.dma_start(
                qpt_sb[:, k * 2 * I : (k + 1) * 2 * I], qpt[k * 128 : (k + 1) * 128, :]
            )

        # Persistent per-core accumulators: unnormalized dots + sumexp matrix.
        MSZ = [128, 128, TQ]
        outsb = [
            sb.tile([msz, NB], F32, tag=f"out{mi}", bufs=1, name=f"outsb{mi}")
            for mi, msz in enumerate(MSZ)
        ]
        semat = [
            sb.tile([msz, NB], F32, tag=f"se{mi}", bufs=1, name=f"semat{mi}")
            for mi, msz in enumerate(MSZ)
        ]

        def load_x(g):
            # One DMA per k-chunk PAIR: img[g, 2j:2j+2] is contiguous, the
            # destination holds chunk 2j in cols 0:N and 2j+1 in cols N:2N.
            xs = []
            for j in range(KC // 2):
                x = sb.tile([128, 2 * N], BF16, tag=f"x{j}", bufs=4, name=f"x{j}g{g}")
                nc.sync.dma_start(
                    x[:].rearrange("p (k n) -> p k n", k=2),
                    img[g, 2 * j : 2 * j + 2].rearrange("k p n -> p k n"),
                )
                xs.append(x)
            return [xs[k // 2][:, (k % 2) * N : (k % 2 + 1) * N] for k in range(KC)]

        def norm_pair(p, xs_a, xs_b):
            # One norm chain for groups 2p (xs_a) and 2p+1 (xs_b):
            # fp16 squares -> per-group add tree -> shared [128, 1024] tile ->
            # one gpsimd partition all-reduce -> rn = Exp(-0.5 * Ln(n2)).
            ssq = sb.tile([128, 2 * N], F16, tag="ssq", bufs=2, name=f"ssqp{p}")
            rns = []
            for half, xs in enumerate((xs_a, xs_b)):
                sq = []
                for k in range(KC):
                    s = sb.tile(
                        [128, N], F16, tag=f"sq{k % 4}", bufs=3, name=f"sqp{p}h{half}k{k}"
                    )
                    if k < SQ_ON_ACT:
                        nc.scalar.square(s[:], xs[k])
                    else:
                        nc.vector.tensor_mul(s[:], xs[k], xs[k])
                    sq.append(s)
                lvl, li = sq, 0
                while len(lvl) > 2:
                    nxt = []
                    for j in range(0, len(lvl), 2):
                        t = sb.tile(
                            [128, N], F16, tag=f"ss{li}{j}", bufs=2,
                            name=f"ssp{p}h{half}l{li}j{j}",
                        )
                        eng = nc.gpsimd if (li == 0 and j == 0) else nc.vector
                        eng.tensor_add(t[:], lvl[j][:], lvl[j + 1][:])
                        nxt.append(t)
                    lvl, li = nxt, li + 1
                nc.vector.tensor_add(
                    ssq[:, half * N : (half + 1) * N], lvl[0][:], lvl[1][:]
                )
                n2 = sb.tile([128, N], F32, tag="n2", bufs=3, name=f"n2p{p}h{half}")
                nc.gpsimd.partition_all_reduce(
                    n2[:],
                    ssq[:, half * N : (half + 1) * N],
                    channels=128,
                    reduce_op=bass_isa.ReduceOp.add,
                )
                # rn = n2^(-1/2); Ln/Exp/Square share one ACT set.
                lnt = sb.tile([128, N], F32, tag="lnt", bufs=3, name=f"lntp{p}h{half}")
                nc.scalar.activation(lnt[:], n2[:], LN)
                rn = sb.tile([128, N], F32, tag="rn", bufs=3, name=f"rnp{p}h{half}")
                nc.scalar.activation(rn[:], lnt[:], EXP, scale=-0.5)
                rns.append(rn)
            return rns

        def mm_chunk(g, xs, coff, msz, nm):
            a = ps.tile([msz, N], F32, tag="sps", bufs=8, name=f"ps{nm}g{g}")
            for k in range(KC):
                nc.tensor.matmul(
                    a[:],
                    qpt_sb[:, k * 2 * I + coff : k * 2 * I + coff + msz],
                    xs[k],
                    start=(k == 0),
                    stop=(k == KC - 1),
                )
            return a

        def softmax_dot(g, mi, sqs, sps, msz):
            # sqs: scaled Q-side logits [msz, N]; sps: scaled P-side [msz, N].
            E = sb.tile([msz, N], F32, tag="E", bufs=2, name=f"Eg{g}m{mi}")
            for h in range(2):
                nc.scalar.activation(
                    E[:, h * R : (h + 1) * R],
                    sqs[:, h * R : (h + 1) * R],
                    EXP,
                    accum_out=semat[mi][:msz, 2 * g + h : 2 * g + h + 1],
                )
            scr = sb.tile([msz, R], F32, tag="scr", bufs=2, name=f"scrg{g}m{mi}")
            for h in range(2):
                nc.vector.scalar_tensor_tensor(
                    out=scr[:],
                    in0=E[:, h * R : (h + 1) * R],
                    scalar=1.0,
                    in1=sps[:, h * R : (h + 1) * R],
                    op0=MULT,
                    op1=MULT,
                    accum_out=outsb[mi][:msz, 2 * g + h : 2 * g + h + 1],
                )

        def main_group(g, xs, rn):
            # Two full 128-row chunk pairs.
            for mi in range(2):
                qa = mm_chunk(g, xs, MCH_Q[mi], 128, f"q{mi}")
                pa = mm_chunk(g, xs, MCH_P[mi], 128, f"p{mi}")
                sqs = sb.tile([128, N], F32, tag="sqs", bufs=2, name=f"sqsg{g}m{mi}")
                nc.vector.tensor_mul(sqs[:], qa[:], rn[:, :])
                sps = sb.tile([128, N], F32, tag="spss", bufs=2, name=f"spsg{g}m{mi}")
                nc.vector.tensor_mul(sps[:], pa[:], rn[:, :])
                softmax_dot(g, mi, sqs, sps, 128)
            # Packed tail: Q rows 256:312 at psum partitions 0:56, P rows at 56:112.
            ta = mm_chunk(g, xs, MCH_T, 2 * TQ, "t")
            ts = sb.tile([2 * TQ, N], F32, tag="tss", bufs=2, name=f"tsg{g}")
            nc.vector.tensor_mul(ts[:], ta[:], rn[: 2 * TQ, :])
            # Shift the P half down to partitions 0:56 (DMA, split over 2 queues).
            tp = sb.tile([TQ, N], F32, tag="tps", bufs=2, name=f"tpg{g}")
            hh = TQ // 2
            nc.sync.dma_start(tp[:hh, :], ts[TQ : TQ + hh, :])
            nc.sync.dma_start(tp[hh:, :], ts[TQ + hh :, :])
            softmax_dot(g, 2, ts[:TQ, :], tp[:], TQ)

        NP = G // 2  # pairs of groups
        xs = {0: load_x(0), 1: load_x(1)}
        rn_pair = {0: norm_pair(0, xs[0], xs[1])}
        for p in range(NP):
            if p + 1 < NP:
                xs[2 * p + 2] = load_x(2 * p + 2)
                xs[2 * p + 3] = load_x(2 * p + 3)
                rn_pair[p + 1] = norm_pair(p + 1, xs[2 * p + 2], xs[2 * p + 3])
            main_group(2 * p, xs.pop(2 * p), rn_pair[p][0][:])
            main_group(2 * p + 1, xs.pop(2 * p + 1), rn_pair.pop(p)[1][:])

        # Final softmax normalization + store.
        offs = [0, 128, 256]
        for mi, msz in enumerate(MSZ):
            rec = sb.tile([msz, NB], F32, tag=f"rec{mi}", bufs=1, name=f"rec{mi}")
            nc.vector.reciprocal(rec[:], semat[mi][:])
            fin = sb.tile([msz, NB], F32, tag=f"fin{mi}", bufs=1, name=f"fin{mi}")
            nc.vector.tensor_mul(fin[:], outsb[mi][:], rec[:])
            nc.sync.dma_start(out[offs[mi] : offs[mi] + msz, :], fin[:])

    nc.compile()
    return nc


def _prepare(inputs):
    img = np.asarray(inputs["img"], np.float32)
    V = np.asarray(inputs["V"], np.float32)
    W1 = np.asarray(inputs["W1"], np.float32)
    W2 = np.asarray(inputs["W2"], np.float32)
    B, Cf, H, W = img.shape
    assert (B, Cf, H * W) == (N_CORES * NB, CF, R), img.shape

    import ml_dtypes

    vv = V.astype(np.float64)
    vv /= np.maximum(np.sqrt((vv * vv).sum(1, keepdims=True)), 1e-12)
    Q = vv @ W1.astype(np.float64)  # [I, CF]
    P = vv @ W2.astype(np.float64)
    # Column order: Q[0:128], Q[128:256], P[0:128], P[128:256], Q[256:], P[256:]
    stacked = np.concatenate(
        [Q[0:128], Q[128:256], P[0:128], P[128:256], Q[256:I], P[256:I]], axis=0
    )
    qpt = np.ascontiguousarray(stacked.T.astype(ml_dtypes.bfloat16))  # [CF, 624]

    # Per-core img: [G, KC, 128, 2*R] bf16 so each (group, k-chunk) x-tile is
    # one contiguous DMA with both batches of the group side by side.
    imgb = img.reshape(B, Cf, H * W).astype(ml_dtypes.bfloat16)
    imgb = imgb.reshape(N_CORES, G, 2, KC, 128, R).transpose(0, 1, 3, 4, 2, 5)
    imgb = np.ascontiguousarray(imgb.reshape(N_CORES, G, KC, 128, 2 * R))
    in_maps = [{"img": imgb[c], "qpt": qpt} for c in range(N_CORES)]
    return in_maps


def run(inputs, **spmd_kwargs):
    """Run the kernel; returns (full_output [B, I], BassKernelResults)."""
    global _PROGRAM
    if _PROGRAM is None:
        _PROGRAM = _build_program()
    from concourse.bass_utils import run_bass_kernel_spmd

    in_maps = _prepare(inputs)
    res = run_bass_kernel_spmd(
        _PROGRAM, in_maps, core_ids=list(range(N_CORES)), **spmd_kwargs
    )
    out = np.concatenate(
        [np.asarray(res.results[c]["out"]).T for c in range(N_CORES)], axis=0
    )
    return np.ascontiguousarray(out, np.float32), res


def kernel(**inputs) -> np.ndarray:
    return run(inputs)[0]
